# revision 12
# baseline (speedup 1.0000x reference)
"""HardBinaryVote Trainium2 kernel — 3-bit packed votes + col-tiled PE.

out[s] = (sum_m w[m]*votes[m,s] > sum_m w[m]/2) as int32, votes in {0,1}.

Design (8 NeuronCores, sample-sharded):
  - Host packs each model TRIPLE (3i, 3i+1, 3i+2) = (a,b,c) into one fp8
    byte 0x38 | (a + 2b + 4c): a normal e4m3 with fixed exponent, value
    1 + a/8 + b/4 + c/2 — AFFINE in the three vote bits. 21 bytes/sample
    (63 = 21*3 exactly): HBM traffic ~5.5 MB/core (vs 15.75 at fp8/vote).
  - Device derives two more affine channels with single-pass DVE bitwise
    ANDs on the raw bytes (uint32 view, 2-port perf mode):
        v1 = B & 0x39 -> value 1 + a/8      v2 = B & 0x3A -> value 1 + b/4
    Then wa*a + wb*b + wc*c = l0*v0 + l1*v1 + l2*v2 - const with
    l0 = 2*wc, l1 = 8*wa - l0, l2 = 4*wb - l0 (fp16, compensated), and
    const + the expected quantization residual folded into the threshold.
  - PE: 4x column tiling (128x32 tiles), each tile streaming its own rhs
    concurrently (~4 cols/cycle). Each rhs column = 6 samples x 21 rows.
    Supergroup: 4 tiles x 5 slots x 3 channels = 60 matmuls accumulate one
    [128,512] PSUM bank; output partition = 32t + 6j + r. 4 full
    supergroups + one 1-slot mini supergroup cover 258048 >= 250000
    samples per core.
  - Thresholds run on the scalar engine: Sign(S - T') -> int8 {-1,0,1};
    host maps > 0. DVE does only the AND passes. PE warmup matmuls on
    zeroed scratch keep HAM at full clock while the first chunks stream.
"""

import sys

import numpy as np

sys.path.insert(0, "/opt/trn_rl_repo")

import ml_dtypes  # noqa: E402

from concourse import bacc, bass_utils, mybir, tile  # noqa: E402

N_MODELS = 63
N_SAMPLES = 2_000_000
N_CORES = 8
S_CORE = N_SAMPLES // N_CORES  # 250000

T = 4  # PE column tiles
J = 5  # slots per tile per full supergroup (windows: 6j + r <= 29)
R = 6  # samples per rhs column (21 rows each)
COLS = 512
G = 4  # full supergroups
CHUNK = J * COLS  # 2560 cols per (g, t) DMA chunk
NCOL_FULL = G * T * CHUNK  # 40960
NCOL_MINI = T * COLS  # 2048 (one slot per tile)
NCOL_TOT = NCOL_FULL + NCOL_MINI  # 43008
S_PAD = NCOL_TOT * R  # 258048 padded samples per core
NWARM = 44

E4 = ml_dtypes.float8_e4m3fn
M1 = 0x39393939
M2 = 0x3A3A3A3A

_last_results = None


def _build_program(threshold: float):
    nc = bacc.Bacc("TRN2", target_bir_lowering=False, debug=False)

    votes_d = nc.dram_tensor(
        "votes", [17, 128, CHUNK], mybir.dt.float8e4, kind="ExternalInput"
    )
    wp_d = nc.dram_tensor("wp", [128, 3, J, 32], mybir.dt.float16, kind="ExternalInput")
    out_d = nc.dram_tensor(
        "out", [G + 1, 128, COLS], mybir.dt.int8, kind="ExternalOutput"
    )

    with tile.TileContext(nc) as tc:
        with (
            tc.tile_pool(name="w", bufs=1) as wpool,
            tc.tile_pool(name="v", bufs=17) as vpool,
            tc.tile_pool(name="p", bufs=14) as ppool,
            tc.tile_pool(name="o", bufs=G + 1) as opool,
            tc.tile_pool(name="ps", bufs=4, space="PSUM") as pspool,
        ):
            # ---- phase 1: all DMA issues (inputs spread over 3 queues) ----
            wt = wpool.tile([128, 3, J, 32], mybir.dt.float16, tag="wp")
            nc.gpsimd.dma_start(out=wt[:], in_=wp_d[:])

            # mini chunk first (smallest -> PE starts earliest); scalar ring
            # carries only early chunks so output DMAs at the end don't queue
            # behind late input packets.
            vt_mini = vpool.tile([128, NCOL_MINI], mybir.dt.float8e4)
            nc.sync.dma_start(out=vt_mini[:], in_=votes_d[16, :, :NCOL_MINI])
            ring3 = [nc.gpsimd, nc.scalar, nc.sync]
            ring2 = [nc.gpsimd, nc.sync]
            vts = {}
            qi = 0
            for g in range(G):
                for t in range(T):
                    vt = vpool.tile([128, CHUNK], mybir.dt.float8e4)
                    q = ring3[qi % 3] if qi < 12 else ring2[qi % 2]
                    q.dma_start(out=vt[:], in_=votes_d[g * T + t])
                    qi += 1
                    vts[(g, t)] = vt

            # ---- PE warmup on zeroed scratch (keeps HAM at K=8/8) ----
            sw = wpool.tile([128, 160], mybir.dt.float8e4, tag="warm")
            nc.vector.memset(sw[:], 0)
            bt = wpool.tile([128, 1], mybir.dt.float32, tag="bias")
            nc.vector.memset(bt[:], -float(threshold))
            ps_w = pspool.tile([128, COLS], mybir.dt.float32)
            for _ in range(NWARM):
                nc.tensor.matmul(
                    ps_w[0:32, 0:128],
                    sw[:, :32],
                    sw[:, 32:160],
                    start=True,
                    stop=True,
                    tile_position=(0, 0),
                )

            # ---- phase 2: parity channels, matmuls, thresholds ----
            def parity(vt, ncol):
                p1 = ppool.tile([128, ncol], mybir.dt.float8e4)
                p2 = ppool.tile([128, ncol], mybir.dt.float8e4)
                for pt, mask in ((p1, M1), (p2, M2)):
                    nc.vector.tensor_scalar(
                        out=pt[:].bitcast(mybir.dt.uint32),
                        in0=vt[:].bitcast(mybir.dt.uint32),
                        scalar1=mask,
                        scalar2=None,
                        op0=mybir.AluOpType.bitwise_and,
                    )
                return p1, p2

            pm1, pm2 = parity(vt_mini, NCOL_MINI)
            chans = {}
            for g in range(G):
                for t in range(T):
                    vt = vts[(g, t)]
                    p1, p2 = parity(vt, CHUNK)
                    chans[(g, t)] = (vt, p1, p2)

            def supergroup(g, nslots, srcs_of):
                ps = pspool.tile([128, COLS], mybir.dt.float32)
                for j in range(nslots):
                    for t in range(T):
                        srcs = srcs_of(t)
                        for ch in range(3):
                            nc.tensor.matmul(
                                ps[32 * t : 32 * t + 32, :],
                                wt[:, ch, j],
                                srcs[ch][:, j * COLS : (j + 1) * COLS],
                                start=(j == 0 and ch == 0),
                                stop=(j == nslots - 1 and ch == 2),
                                tile_position=(0, 32 * t),
                            )
                ot = opool.tile([128, COLS], mybir.dt.int8)
                nc.scalar.activation(
                    out=ot[:],
                    in_=ps[:],
                    func=mybir.ActivationFunctionType.Sign,
                    bias=bt[:],
                )
                nc.scalar.dma_start(out=out_d[g], in_=ot[:])

            supergroup(
                G,
                1,
                lambda t: tuple(
                    x[:, t * COLS : (t + 1) * COLS] for x in (vt_mini, pm1, pm2)
                ),
            )
            for g in range(G):
                supergroup(g, J, lambda t, g=g: chans[(g, t)])

    nc.compile()
    return nc


def _pack_weights(w: np.ndarray):
    """Returns (wp [128,3,J,32] fp16, threshold offset const+mean_err)."""
    w = w.astype(np.float64)
    wa, wb, wc = w[0::3], w[1::3], w[2::3]
    l0 = (2 * wc).astype(np.float16)
    l0d = l0.astype(np.float64)
    l1 = (8 * wa - l0d).astype(np.float16)
    l2 = (4 * wb - l0d).astype(np.float16)
    l1d, l2d = l1.astype(np.float64), l2.astype(np.float64)

    const = (l0d + l1d + l2d).sum()
    ea = (l0d + l1d) / 8 - wa
    eb = (l0d + l2d) / 4 - wb
    ec = l0d / 2 - wc
    mean_err = 0.5 * (ea.sum() + eb.sum() + ec.sum())

    wp = np.zeros((128, 3, J, 32), np.float16)
    lam = (l0, l1, l2)
    for ch in range(3):
        for j in range(J):
            for r in range(R):
                wp[21 * r : 21 * r + 21, ch, j, 6 * j + r] = lam[ch]
    return wp, const + mean_err


def kernel(votes: np.ndarray, vote_weights: np.ndarray) -> np.ndarray:
    global _last_results
    votes = np.ascontiguousarray(votes, dtype=np.int32)
    w = np.asarray(vote_weights, dtype=np.float32)
    assert votes.shape == (N_MODELS, N_SAMPLES)

    wp, t_off = _pack_weights(w)
    threshold = float(
        np.float64(np.float32(w.astype(np.float64).sum() / 2.0)) + t_off
    )

    v8 = votes.astype(np.uint8)
    packed = (0x38 | (v8[0::3] + 2 * v8[1::3] + 4 * v8[2::3])).astype(np.uint8)

    in_maps = []
    for c in range(N_CORES):
        sh = np.zeros((21, S_PAD), np.uint8)
        sh[:, :S_CORE] = packed[:, c * S_CORE : (c + 1) * S_CORE]
        # sample s -> (col = s//6, r = s%6); partition p = 21r + row
        dev = np.zeros((128, NCOL_TOT), np.uint8)
        dev[:126] = sh.reshape(21, NCOL_TOT, R).transpose(2, 0, 1).reshape(126, -1)
        # chunk-major DRAM image: each chunk contiguous for HBM locality
        cm = np.zeros((17, 128, CHUNK), np.uint8)
        cm[:16] = dev[:, :NCOL_FULL].reshape(128, 16, CHUNK).transpose(1, 0, 2)
        cm[16, :, :NCOL_MINI] = dev[:, NCOL_FULL:]
        in_maps.append({"votes": np.ascontiguousarray(cm).view(E4), "wp": wp})

    nc = _build_program(threshold)
    res = bass_utils.run_bass_kernel_spmd(nc, in_maps, core_ids=list(range(N_CORES)))
    _last_results = res

    outs = []
    for c in range(N_CORES):
        O = res.results[c]["out"]  # [G+1, 128, 512] int8 sign values
        full = (O[:G].reshape(G, T, 32, COLS)[:, :, :30, :] > 0).astype(np.int32)
        full = full.reshape(G, T, J, R, COLS).transpose(0, 1, 2, 4, 3).reshape(-1)
        mini = (O[G].reshape(T, 32, COLS)[:, :R, :] > 0).astype(np.int32)
        mini = mini.transpose(0, 2, 1).reshape(-1)
        outs.append(np.concatenate([full, mini])[:S_CORE])
    return np.ascontiguousarray(np.concatenate(outs).astype(np.int32))
